# revision 15
# baseline (speedup 1.0000x reference)
"""Causal self-attention (B=4, T=2048, C=1024, H=16) on 8 TRN2 NeuronCores.

Sharding: core = (batch, head-group) — data parallel over the 4 batches,
tensor parallel over 2 groups of 8 heads (Megatron-style column/row split of
the qkv / out projections).  Each core computes a [T, C] partial of the out
projection for its head group; the host sums the two partials per batch and
adds b_out, so no device collectives are needed.

v2 (vs v1 baseline at ~630us):
  * All matmul operands are bf16 (PSUM accumulation stays fp32).  Same PE
    streaming rate as f32r (1 cycle/row) but enables Fast Weight Load
    (v1 spent 253us in serialized fp32 LDWEIGHTS), removes the f32r
    moving-dim<256 4x penalty, and halves DMA + SBUF footprint.  Host casts
    inputs to bf16.
  * One exp ACTIVATE per tk-block covering BOTH heads of a pair via a
    [128, 2, 512] PSUM tile spanning 2 banks (v1: 320 exps -> 160, less
    fixed per-instruction overhead on ScalarE).
  * Softmax normalization: v1 burned 107us of DVE in single-partition
    5-pass RECIPROCALs.  Now: copy the two denominator rows (PSUM row D)
    to partitions 0/1, one reciprocal_approx_fast on [2,512], one DRAM
    bounce DMA broadcasting both heads' 1/den to [64, 2, 512], then one
    tensor_mul per head.
  * Loop body stays slab-interleaved (projections / attention / out-proj)
    so the Tile scheduler can fill PE gaps during ScalarE exp latency with
    next-slab projection matmuls — keeping the PE HAM-warm at 2.4 GHz
    (v1 ran 67% of the time at the 1.2 GHz throttle).
"""

import os
import sys
from contextlib import ExitStack

import numpy as np

for _p in ("/opt/trn_rl_repo", "/root/.axon_site/_ro/trn_rl_repo"):
    if os.path.isdir(_p) and _p not in sys.path:
        sys.path.append(_p)

import concourse.bacc as bacc
import concourse.bass as bass
import concourse.tile as tile
from concourse import mybir
from concourse.bass_utils import run_bass_kernel_spmd
from concourse.masks import make_upper_triangular

AF = mybir.ActivationFunctionType
ALU = mybir.AluOpType
F32 = mybir.dt.float32
BF16 = mybir.dt.bfloat16

P = 128
SLAB = 512

B, T, C, H, D = 4, 2048, 1024, 16, 64
N_CORES = 8
N_GROUPS = 2          # head groups (tensor-parallel degree per batch)
HL = H // N_GROUPS    # heads per core
CL = HL * D           # local qkv width


def _build_nc():
    NCK = C // P
    MQK = 2 * CL // P
    MQ = MQK // 2
    TT = T // P
    NS = T // SLAB
    YC = CL // P
    W_OUT = min(SLAB, C)
    NOUT = C // W_OUT
    scale = 1.0 / np.sqrt(D)

    nc = bacc.Bacc("TRN2", target_bir_lowering=False, debug=False,
                   num_devices=N_CORES)
    xT = nc.dram_tensor("xT", [C, T], BF16, kind="ExternalInput")
    wqk = nc.dram_tensor("wqk", [C, 2 * CL], BF16, kind="ExternalInput")
    wv = nc.dram_tensor("wv", [C, CL], BF16, kind="ExternalInput")
    wout = nc.dram_tensor("wout", [CL, C], BF16, kind="ExternalInput")
    bqk = nc.dram_tensor("bqk", [P, MQK], F32, kind="ExternalInput")
    bv = nc.dram_tensor("bv", [1, CL], BF16, kind="ExternalInput")
    outp = nc.dram_tensor("outp", [T, C], BF16, kind="ExternalOutput")
    scr = nc.dram_tensor("scr", [2 * HL // 2 * NS, SLAB], F32)

    with tile.TileContext(nc) as tc, ExitStack() as ctx:
        pool = lambda name, bufs, **kw: ctx.enter_context(
            tc.tile_pool(name=name, bufs=bufs, **kw))

        const = pool("const", 1)
        kp = pool("kp", 1)
        vp = pool("vp", 1)
        wqkp = pool("wqkp", 1)
        wvp = pool("wvp", 1)
        woutp = pool("woutp", 1)
        xp = pool("xp", 1)
        qp = pool("qp", 2)
        yTp = pool("yTp", 2)
        expp = pool("expp", 3)
        yrawp = pool("yrawp", 2)
        den2p = pool("den2p", 2)
        recbp = pool("recbp", 2)
        y8p = pool("y8", 2)
        otp = pool("ot", 2)
        psS = pool("psS", 2, space="PSUM")      # [P,2,SLAB] pair tiles: 4 banks
        psY = pool("psY", 1, space="PSUM")      # py0+py1: 2 banks
        psPO = pool("psPO", 2, space="PSUM")    # shared proj/out evict: 2 banks

        k_sb = kp.tile([P, MQ, T], BF16)
        v_sb = vp.tile([P, TT, HL, D + 1], BF16)
        x_sb = xp.tile([P, NCK, T], BF16)
        wqk_sb = wqkp.tile([P, NCK, 2 * CL], BF16)
        wv_sb = wvp.tile([P, NCK, CL], BF16)
        wout_sb = woutp.tile([P, YC, C], BF16)
        bqk_sb = const.tile([P, MQK], F32)
        bv_sb = const.tile([1, CL], BF16)
        mask01 = const.tile([P, P], BF16)
        maskf = const.tile([P, P], F32)
        ones_row = const.tile([1, P], BF16)

        # Preload: sync queue carries what phase A(0) needs (wqk + x),
        # interleaved so chunk 0 of each lands early; x is loaded whole
        # (contiguous 4KB rows — per-slab [128,512] slices would be 1KB
        # strided rows, which the DMA engines move ~4x slower).  Everything
        # else goes through the gpsimd SWDGE queue so the ~0.6us per-DMA
        # issue cost doesn't serialize ahead of the first matmuls.
        for c in range(NCK):
            nc.sync.dma_start(out=wqk_sb[:, c, :], in_=wqk[c * P:(c + 1) * P, :])
            nc.sync.dma_start(out=x_sb[:, c, :], in_=xT[c * P:(c + 1) * P, :])
        nc.gpsimd.dma_start(out=bqk_sb[:, :], in_=bqk[:, :])
        nc.gpsimd.dma_start(out=bv_sb[:, :], in_=bv[:, :])
        for c in range(NCK):
            nc.gpsimd.dma_start(out=wv_sb[:, c, :], in_=wv[c * P:(c + 1) * P, :])
        for c in range(YC):
            nc.gpsimd.dma_start(out=wout_sb[:, c, :],
                                in_=wout[c * P:(c + 1) * P, :])
        # mask01[p, f] = 1 if f >= p else 0  (S^T visibility: tq >= tk);
        # built in f32 then cast (affine_select path is f32).
        make_upper_triangular(nc, maskf[:, :], val=1.0, diag=True)
        nc.vector.tensor_copy(mask01[:, :], maskf[:, :])
        nc.vector.memset(ones_row[:, :], 1.0)
        nc.vector.memset(v_sb[:, :, :, D], 1.0)

        for s in range(NS):
            t0 = s * SLAB
            # ---- A(s): projections for this slab ----
            q_sb = qp.tile([P, MQ, SLAB], BF16)
            for m in range(MQK):
                ps = psPO.tile([P, SLAB], F32, tag="po")
                for c in range(NCK):
                    nc.tensor.matmul(
                        ps[:, :],
                        wqk_sb[:, c, m * P:(m + 1) * P],
                        x_sb[:, c, t0:t0 + SLAB],
                        start=(c == 0), stop=(c == NCK - 1))
                dst = (q_sb[:, m, :] if m < MQ
                       else k_sb[:, m - MQ, t0:t0 + SLAB])
                sc = scale if m < MQ else 1.0
                nc.vector.tensor_scalar(
                    dst, ps[:, :], sc, bqk_sb[:, m:m + 1],
                    op0=ALU.mult, op1=ALU.add)
            for sub in range(SLAB // P):
                tt = s * (SLAB // P) + sub
                ps = psPO.tile([P, CL], F32, tag="po")
                for c in range(NCK):
                    nc.tensor.matmul(
                        ps[:, :],
                        x_sb[:, c, tt * P:(tt + 1) * P],
                        wv_sb[:, c, :],
                        start=(c == 0), stop=False)
                nc.tensor.matmul(
                    ps[:, :], ones_row[:, :],
                    bv_sb[0:1, :], start=False, stop=True)
                nc.vector.tensor_copy(
                    v_sb[:, tt, :, 0:D],
                    ps[:, :].rearrange("p (h d) -> p h d", d=D))

            # ---- B(s): attention; even/odd head pairs share the PE array
            # via tile_position row groups (concurrent K=64).  One exp per
            # tk-block covers both heads ([P, 2, SLAB] across 2 PSUM banks).
            yT_sb = yTp.tile([P, YC, SLAB], BF16)
            for hp in range(HL // 2):
                nblk = (s + 1) * SLAB // P
                py0 = psY.tile([D + 1, SLAB], F32, tag="py0")
                py1 = psY.tile([D + 1, SLAB], F32, tag="py1")
                pys = (py0, py1)
                for b in range(nblk):
                    tk0 = b * P
                    off = tk0 - t0
                    vis = max(0, off)
                    ps3 = psS.tile([P, 2, SLAB], F32)
                    for i in range(2):
                        row0 = i * 64
                        nc.tensor.matmul(
                            ps3[:, i, vis:SLAB],
                            k_sb[row0:row0 + 64, hp, tk0:tk0 + P],
                            q_sb[row0:row0 + 64, hp, vis:SLAB],
                            start=True, stop=True,
                            tile_position=(row0, 0))
                    ep3 = expp.tile([P, 2, SLAB], BF16)
                    nc.scalar.activation(ep3[:, :, vis:SLAB],
                                         ps3[:, :, vis:SLAB], AF.Exp)
                    if off >= 0:
                        for i in range(2):
                            nc.gpsimd.tensor_mul(
                                ep3[:, i, off:off + P],
                                ep3[:, i, off:off + P], mask01[:, :])
                    for i in range(2):
                        nc.tensor.matmul(
                            pys[i][0:D + 1, vis:SLAB],
                            v_sb[:, b, 2 * hp + i, 0:D + 1],
                            ep3[:, i, vis:SLAB],
                            start=(b == 0), stop=(b == nblk - 1))
                # Evict py0/py1 RAW to SBUF immediately (ScalarE + DVE in
                # parallel) so the psY banks free up and the next pair's PV
                # can start; the whole normalize chain then runs from SBUF
                # off the PE critical path.  (reciprocal_approx_fast needs
                # an SBUF source — PSUM reads feed it garbage.)
                yraw0 = yrawp.tile([D + 1, SLAB], F32, tag="yraw0")
                yraw1 = yrawp.tile([D + 1, SLAB], F32, tag="yraw1")
                nc.scalar.copy(yraw0[:, :], py0[:, :])
                nc.vector.tensor_copy(yraw1[:, :], py1[:, :])
                yraws = (yraw0, yraw1)
                sidx = 2 * (hp * NS + s)
                # Bounce the RAW denominator rows through DRAM to broadcast
                # them across partitions, then reciprocal on the broadcast
                # tile (SBUF, partition base 0 — the only layout the custom
                # DVE reciprocal handles).
                nc.sync.dma_start(out=scr[sidx:sidx + 1, :],
                                  in_=yraw0[D:D + 1, :])
                nc.sync.dma_start(out=scr[sidx + 1:sidx + 2, :],
                                  in_=yraw1[D:D + 1, :])
                src = scr[sidx:sidx + 2, :]
                bsrc = bass.AP(tensor=src.tensor, offset=src.offset,
                               ap=[[0, 64], [SLAB, 2], [1, SLAB]])
                denb = recbp.tile([64, 2, SLAB], F32, tag="denb")
                recb = recbp.tile([64, 2, SLAB], F32, tag="recb")
                nc.sync.dma_start(out=denb[:, :, :], in_=bsrc)
                nc.vector.reciprocal_approx_fast(recb[:, :, :],
                                                 denb[:, :, :])
                y8 = y8p.tile([P, SLAB], BF16)
                for i in range(2):
                    nc.vector.tensor_mul(y8[i * 64:(i + 1) * 64, :],
                                         yraws[i][0:D, :], recb[:, i, :])
                nc.sync.dma_start(out=yT_sb[:, hp, :], in_=y8[:, :])

            # ---- C(s): out projection for this slab ----
            for sub in range(SLAB // P):
                for n in range(NOUT):
                    n0 = n * W_OUT
                    ps = psPO.tile([P, W_OUT], F32, tag="po")
                    for c in range(YC):
                        nc.tensor.matmul(
                            ps[:, :],
                            yT_sb[:, c, sub * P:(sub + 1) * P],
                            wout_sb[:, c, n0:n0 + W_OUT],
                            start=(c == 0), stop=(c == YC - 1))
                    ot = otp.tile([P, W_OUT], BF16)
                    nc.vector.tensor_copy(ot[:, :], ps[:, :])
                    nc.sync.dma_start(
                        out=outp[t0 + sub * P:t0 + (sub + 1) * P,
                                 n0:n0 + W_OUT],
                        in_=ot[:, :])

    nc.compile()
    return nc


_NC_CACHE = None


def _get_nc():
    global _NC_CACHE
    if _NC_CACHE is None:
        _NC_CACHE = _build_nc()
    return _NC_CACHE


def make_in_maps(x, W_qkv, b_qkv, W_out):
    bf16 = mybir.dt.np(BF16)
    scale = 1.0 / np.sqrt(D)
    MQK = 2 * CL // P
    in_maps = []
    for core in range(N_CORES):
        b, hg = divmod(core, N_GROUPS)
        qs = slice(hg * CL, (hg + 1) * CL)
        ks = slice(C + hg * CL, C + (hg + 1) * CL)
        vs = slice(2 * C + hg * CL, 2 * C + (hg + 1) * CL)
        bqk_cat = np.concatenate([b_qkv[qs] * scale, b_qkv[ks]])
        in_maps.append({
            "xT": np.ascontiguousarray(x[b].T).astype(bf16),
            "wqk": np.ascontiguousarray(
                np.concatenate([W_qkv[:, qs], W_qkv[:, ks]],
                               axis=1)).astype(bf16),
            "wv": np.ascontiguousarray(W_qkv[:, vs]).astype(bf16),
            "wout": np.ascontiguousarray(W_out[hg * CL:(hg + 1) * CL,
                                               :]).astype(bf16),
            "bqk": np.ascontiguousarray(bqk_cat.reshape(MQK, P).T),
            "bv": np.ascontiguousarray(b_qkv[vs].reshape(1, CL)).astype(bf16),
        })
    return in_maps


def kernel(x, W_qkv, b_qkv, W_out, b_out):
    x = np.asarray(x, dtype=np.float32)
    W_qkv = np.asarray(W_qkv, dtype=np.float32)
    b_qkv = np.asarray(b_qkv, dtype=np.float32)
    W_out = np.asarray(W_out, dtype=np.float32)
    b_out = np.asarray(b_out, dtype=np.float32)

    nc = _get_nc()
    in_maps = make_in_maps(x, W_qkv, b_qkv, W_out)
    res = run_bass_kernel_spmd(nc, in_maps, core_ids=list(range(N_CORES)))

    out = np.empty((B, T, C), dtype=np.float32)
    for b in range(B):
        out[b] = (res.results[N_GROUPS * b]["outp"].astype(np.float32)
                  + res.results[N_GROUPS * b + 1]["outp"].astype(np.float32)
                  + b_out)
    return out


# revision 18
# speedup vs baseline: 1.2095x; 1.2095x over previous
"""Causal self-attention (B=4, T=2048, C=1024, H=16) on 8 TRN2 NeuronCores.

Sharding: core = (batch, head-group) — data parallel over the 4 batches,
tensor parallel over 2 groups of 8 heads (Megatron-style column/row split of
the qkv / out projections).  Each core computes a [T, C] partial of the out
projection for its head group; the host sums the two partials per batch and
adds b_out, so no device collectives are needed.

v2 (vs v1 baseline at ~630us):
  * All matmul operands are bf16 (PSUM accumulation stays fp32).  Same PE
    streaming rate as f32r (1 cycle/row) but enables Fast Weight Load
    (v1 spent 253us in serialized fp32 LDWEIGHTS), removes the f32r
    moving-dim<256 4x penalty, and halves DMA + SBUF footprint.  Host casts
    inputs to bf16.
  * One exp ACTIVATE per tk-block covering BOTH heads of a pair via a
    [128, 2, 512] PSUM tile spanning 2 banks (v1: 320 exps -> 160, less
    fixed per-instruction overhead on ScalarE).
  * Softmax normalization: v1 burned 107us of DVE in single-partition
    5-pass RECIPROCALs.  Now: copy the two denominator rows (PSUM row D)
    to partitions 0/1, one reciprocal_approx_fast on [2,512], one DRAM
    bounce DMA broadcasting both heads' 1/den to [64, 2, 512], then one
    tensor_mul per head.
  * Loop body stays slab-interleaved (projections / attention / out-proj)
    so the Tile scheduler can fill PE gaps during ScalarE exp latency with
    next-slab projection matmuls — keeping the PE HAM-warm at 2.4 GHz
    (v1 ran 67% of the time at the 1.2 GHz throttle).
"""

import os
import sys
from contextlib import ExitStack

import numpy as np

for _p in ("/opt/trn_rl_repo", "/root/.axon_site/_ro/trn_rl_repo"):
    if os.path.isdir(_p) and _p not in sys.path:
        sys.path.append(_p)

import concourse.bacc as bacc
import concourse.bass as bass
import concourse.tile as tile
from concourse import mybir
from concourse.bass_utils import run_bass_kernel_spmd
from concourse.masks import make_upper_triangular

AF = mybir.ActivationFunctionType
ALU = mybir.AluOpType
F32 = mybir.dt.float32
BF16 = mybir.dt.bfloat16

P = 128
SLAB = 512

B, T, C, H, D = 4, 2048, 1024, 16, 64
N_CORES = 8
N_GROUPS = 2          # head groups (tensor-parallel degree per batch)
HL = H // N_GROUPS    # heads per core
CL = HL * D           # local qkv width


def _build_nc():
    NCK = C // P
    MQK = 2 * CL // P
    MQ = MQK // 2
    TT = T // P
    NS = T // SLAB
    YC = CL // P
    W_OUT = min(SLAB, C)
    NOUT = C // W_OUT
    scale = 1.0 / np.sqrt(D)

    nc = bacc.Bacc("TRN2", target_bir_lowering=False, debug=False,
                   num_devices=N_CORES)
    xT = nc.dram_tensor("xT", [C, T], BF16, kind="ExternalInput")
    wqk = nc.dram_tensor("wqk", [C, 2 * CL], BF16, kind="ExternalInput")
    wv = nc.dram_tensor("wv", [C, CL], BF16, kind="ExternalInput")
    wout = nc.dram_tensor("wout", [CL, C], BF16, kind="ExternalInput")
    bqk = nc.dram_tensor("bqk", [P, MQK], F32, kind="ExternalInput")
    bv = nc.dram_tensor("bv", [1, CL], BF16, kind="ExternalInput")
    outp = nc.dram_tensor("outp", [T, C], BF16, kind="ExternalOutput")
    scr = nc.dram_tensor("scr", [2 * HL // 2 * NS, SLAB], BF16)

    with tile.TileContext(nc) as tc, ExitStack() as ctx:
        pool = lambda name, bufs, **kw: ctx.enter_context(
            tc.tile_pool(name=name, bufs=bufs, **kw))

        const = pool("const", 1)
        kp = pool("kp", 1)
        vp = pool("vp", 1)
        wqkp = pool("wqkp", 1)
        wvp = pool("wvp", 1)
        woutp = pool("woutp", 1)
        xp = pool("xp", 1)
        qp = pool("qp", 2)
        yTp = pool("yTp", 2)
        expp = pool("expp", 3)
        yrawp = pool("yrawp", 2)
        den2p = pool("den2p", 2)
        recbp = pool("recbp", 2)
        y8p = pool("y8", 2)
        otp = pool("ot", 2)
        psS = pool("psS", 2, space="PSUM")      # [P,2,SLAB] pair tiles: 4 banks
        psY = pool("psY", 1, space="PSUM")      # py0+py1: 2 banks
        psPO = pool("psPO", 2, space="PSUM")    # shared proj/out evict: 2 banks

        k_sb = kp.tile([P, MQ, T], BF16)
        v_sb = vp.tile([P, TT, HL, D + 1], BF16)
        x_sb = xp.tile([P, NCK, T], BF16)
        wqk_sb = wqkp.tile([P, NCK, 2 * CL], BF16)
        wv_sb = wvp.tile([P, NCK, CL], BF16)
        wout_sb = woutp.tile([P, YC, C], BF16)
        bqk_sb = const.tile([P, MQK], F32)
        bv_sb = const.tile([1, CL], BF16)
        mask01 = const.tile([P, P], BF16)
        maskf = const.tile([P, P], F32)
        ones_row = const.tile([1, P], BF16)

        # Preload: sync queue carries what phase A(0) needs (wqk + x),
        # interleaved so chunk 0 of each lands early; x is loaded whole
        # (contiguous 4KB rows — per-slab [128,512] slices would be 1KB
        # strided rows, which the DMA engines move ~4x slower).  Everything
        # else goes through the gpsimd SWDGE queue so the ~0.6us per-DMA
        # issue cost doesn't serialize ahead of the first matmuls.
        for c in range(NCK):
            nc.sync.dma_start(out=wqk_sb[:, c, :], in_=wqk[c * P:(c + 1) * P, :])
            nc.sync.dma_start(out=x_sb[:, c, :], in_=xT[c * P:(c + 1) * P, :])
        nc.gpsimd.dma_start(out=bqk_sb[:, :], in_=bqk[:, :])
        nc.gpsimd.dma_start(out=bv_sb[:, :], in_=bv[:, :])
        for c in range(NCK):
            nc.gpsimd.dma_start(out=wv_sb[:, c, :], in_=wv[c * P:(c + 1) * P, :])
        for c in range(YC):
            nc.gpsimd.dma_start(out=wout_sb[:, c, :],
                                in_=wout[c * P:(c + 1) * P, :])
        # mask01[p, f] = 1 if f >= p else 0  (S^T visibility: tq >= tk);
        # built in f32 then cast (affine_select path is f32).
        make_upper_triangular(nc, maskf[:, :], val=1.0, diag=True)
        nc.vector.tensor_copy(mask01[:, :], maskf[:, :])
        nc.vector.memset(ones_row[:, :], 1.0)
        nc.vector.memset(v_sb[:, :, :, D], 1.0)

        for s in range(NS):
            t0 = s * SLAB
            # ---- A(s): projections for this slab ----
            q_sb = qp.tile([P, MQ, SLAB], BF16)
            for m in range(MQK):
                ps = psPO.tile([P, SLAB], F32, tag="po")
                for c in range(NCK):
                    nc.tensor.matmul(
                        ps[:, :],
                        wqk_sb[:, c, m * P:(m + 1) * P],
                        x_sb[:, c, t0:t0 + SLAB],
                        start=(c == 0), stop=(c == NCK - 1))
                dst = (q_sb[:, m, :] if m < MQ
                       else k_sb[:, m - MQ, t0:t0 + SLAB])
                sc = scale if m < MQ else 1.0
                nc.vector.tensor_scalar(
                    dst, ps[:, :], sc, bqk_sb[:, m:m + 1],
                    op0=ALU.mult, op1=ALU.add)
            for sub in range(SLAB // P):
                tt = s * (SLAB // P) + sub
                ps = psPO.tile([P, CL], F32, tag="po")
                for c in range(NCK):
                    nc.tensor.matmul(
                        ps[:, :],
                        x_sb[:, c, tt * P:(tt + 1) * P],
                        wv_sb[:, c, :],
                        start=(c == 0), stop=False)
                nc.tensor.matmul(
                    ps[:, :], ones_row[:, :],
                    bv_sb[0:1, :], start=False, stop=True)
                nc.vector.tensor_copy(
                    v_sb[:, tt, :, 0:D],
                    ps[:, :].rearrange("p (h d) -> p h d", d=D))

            # ---- B(s): attention; even/odd head pairs share the PE array
            # via tile_position row groups (concurrent K=64).  One exp per
            # tk-block covers both heads ([P, 2, SLAB] across 2 PSUM banks).
            # C(s-1) out-projection groups are interleaved after each pair:
            # Tile's schedule is STATIC and each engine queue is strict
            # FIFO, so C(s) emitted right after B(s) would block the PE
            # behind the last pair's normalize/bounce chain; C(s-1)'s
            # inputs are long complete and give the PE filler work.
            yT_prev = yT_sb if s > 0 else None
            t0_prev = t0 - SLAB
            yT_sb = yTp.tile([P, YC, SLAB], BF16)
            for hp in range(HL // 2):
                nblk = (s + 1) * SLAB // P
                py0 = psY.tile([D + 1, SLAB], F32, tag="py0")
                py1 = psY.tile([D + 1, SLAB], F32, tag="py1")
                pys = (py0, py1)
                for b in range(nblk):
                    tk0 = b * P
                    off = tk0 - t0
                    vis = max(0, off)
                    ps3 = psS.tile([P, 2, SLAB], F32)
                    for i in range(2):
                        row0 = i * 64
                        nc.tensor.matmul(
                            ps3[:, i, vis:SLAB],
                            k_sb[row0:row0 + 64, hp, tk0:tk0 + P],
                            q_sb[row0:row0 + 64, hp, vis:SLAB],
                            start=True, stop=True,
                            tile_position=(row0, 0))
                    ep3 = expp.tile([P, 2, SLAB], BF16)
                    nc.scalar.activation(ep3[:, :, vis:SLAB],
                                         ps3[:, :, vis:SLAB], AF.Exp)
                    if off >= 0:
                        for i in range(2):
                            nc.gpsimd.tensor_mul(
                                ep3[:, i, off:off + P],
                                ep3[:, i, off:off + P], mask01[:, :])
                    for i in range(2):
                        nc.tensor.matmul(
                            pys[i][0:D + 1, vis:SLAB],
                            v_sb[:, b, 2 * hp + i, 0:D + 1],
                            ep3[:, i, vis:SLAB],
                            start=(b == 0), stop=(b == nblk - 1))
                # Evict py0/py1 RAW to SBUF immediately (ScalarE + DVE in
                # parallel) so the psY banks free up and the next pair's PV
                # can start; the whole normalize chain then runs from SBUF
                # off the PE critical path.  (reciprocal_approx_fast needs
                # an SBUF source — PSUM reads feed it garbage.)
                yraw0 = yrawp.tile([D + 1, SLAB], F32, tag="yraw0")
                yraw1 = yrawp.tile([D + 1, SLAB], F32, tag="yraw1")
                nc.scalar.copy(yraw0[:, :], py0[:, :])
                nc.vector.tensor_copy(yraw1[:, :], py1[:, :])
                yraws = (yraw0, yraw1)
                sidx = 2 * (hp * NS + s)
                # 1/den first (reciprocal_approx_fast needs an SBUF source
                # at partition base 0; ScalarE handles the partition-64
                # row move), cast to bf16, then a DRAM bounce broadcasts
                # both heads' 1/den across partitions in one 128KB DMA.
                den2 = den2p.tile([1, 2 * SLAB], F32, tag="den2")
                rec2 = den2p.tile([1, 2 * SLAB], F32, tag="rec2")
                recc = den2p.tile([1, 2 * SLAB], BF16, tag="recc")
                for i in range(2):
                    nc.scalar.copy(den2[:, i * SLAB:(i + 1) * SLAB],
                                   yraws[i][D:D + 1, :])
                nc.vector.reciprocal_approx_fast(rec2[:, :], den2[:, :])
                nc.vector.tensor_copy(recc[:, :], rec2[:, :])
                nc.sync.dma_start(out=scr[sidx:sidx + 2, :],
                                  in_=recc[:, :])
                src = scr[sidx:sidx + 2, :]
                bsrc = bass.AP(tensor=src.tensor, offset=src.offset,
                               ap=[[0, 64], [SLAB, 2], [1, SLAB]])
                recb = recbp.tile([64, 2, SLAB], BF16, tag="recb")
                nc.sync.dma_start(out=recb[:, :, :], in_=bsrc)
                y8 = y8p.tile([P, SLAB], BF16)
                for i in range(2):
                    nc.vector.tensor_mul(y8[i * 64:(i + 1) * 64, :],
                                         yraws[i][0:D, :], recb[:, i, :])
                nc.sync.dma_start(out=yT_sb[:, hp, :], in_=y8[:, :])

                # ---- C(s-1) filler: two out-projection groups per pair --
                if yT_prev is not None:
                    for g in range(2):
                        sub, n = divmod(2 * hp + g, NOUT)
                        n0 = n * W_OUT
                        ps = psPO.tile([P, W_OUT], F32, tag="po")
                        for c in range(YC):
                            nc.tensor.matmul(
                                ps[:, :],
                                yT_prev[:, c, sub * P:(sub + 1) * P],
                                wout_sb[:, c, n0:n0 + W_OUT],
                                start=(c == 0), stop=(c == YC - 1))
                        ot = otp.tile([P, W_OUT], BF16)
                        nc.vector.tensor_copy(ot[:, :], ps[:, :])
                        nc.sync.dma_start(
                            out=outp[t0_prev + sub * P:
                                     t0_prev + (sub + 1) * P,
                                     n0:n0 + W_OUT],
                            in_=ot[:, :])

        # ---- C(NS-1): final slab's out projection (tail) ----
        t0 = (NS - 1) * SLAB
        for sub in range(SLAB // P):
            for n in range(NOUT):
                n0 = n * W_OUT
                ps = psPO.tile([P, W_OUT], F32, tag="po")
                for c in range(YC):
                    nc.tensor.matmul(
                        ps[:, :],
                        yT_sb[:, c, sub * P:(sub + 1) * P],
                        wout_sb[:, c, n0:n0 + W_OUT],
                        start=(c == 0), stop=(c == YC - 1))
                ot = otp.tile([P, W_OUT], BF16)
                nc.vector.tensor_copy(ot[:, :], ps[:, :])
                nc.sync.dma_start(
                    out=outp[t0 + sub * P:t0 + (sub + 1) * P,
                             n0:n0 + W_OUT],
                    in_=ot[:, :])

    nc.compile()
    return nc


_NC_CACHE = None


def _get_nc():
    global _NC_CACHE
    if _NC_CACHE is None:
        _NC_CACHE = _build_nc()
    return _NC_CACHE


def make_in_maps(x, W_qkv, b_qkv, W_out):
    bf16 = mybir.dt.np(BF16)
    scale = 1.0 / np.sqrt(D)
    MQK = 2 * CL // P
    in_maps = []
    for core in range(N_CORES):
        b, hg = divmod(core, N_GROUPS)
        qs = slice(hg * CL, (hg + 1) * CL)
        ks = slice(C + hg * CL, C + (hg + 1) * CL)
        vs = slice(2 * C + hg * CL, 2 * C + (hg + 1) * CL)
        bqk_cat = np.concatenate([b_qkv[qs] * scale, b_qkv[ks]])
        in_maps.append({
            "xT": np.ascontiguousarray(x[b].T).astype(bf16),
            "wqk": np.ascontiguousarray(
                np.concatenate([W_qkv[:, qs], W_qkv[:, ks]],
                               axis=1)).astype(bf16),
            "wv": np.ascontiguousarray(W_qkv[:, vs]).astype(bf16),
            "wout": np.ascontiguousarray(W_out[hg * CL:(hg + 1) * CL,
                                               :]).astype(bf16),
            "bqk": np.ascontiguousarray(bqk_cat.reshape(MQK, P).T),
            "bv": np.ascontiguousarray(b_qkv[vs].reshape(1, CL)).astype(bf16),
        })
    return in_maps


def kernel(x, W_qkv, b_qkv, W_out, b_out):
    x = np.asarray(x, dtype=np.float32)
    W_qkv = np.asarray(W_qkv, dtype=np.float32)
    b_qkv = np.asarray(b_qkv, dtype=np.float32)
    W_out = np.asarray(W_out, dtype=np.float32)
    b_out = np.asarray(b_out, dtype=np.float32)

    nc = _get_nc()
    in_maps = make_in_maps(x, W_qkv, b_qkv, W_out)
    res = run_bass_kernel_spmd(nc, in_maps, core_ids=list(range(N_CORES)))

    out = np.empty((B, T, C), dtype=np.float32)
    for b in range(B):
        out[b] = (res.results[N_GROUPS * b]["outp"].astype(np.float32)
                  + res.results[N_GROUPS * b + 1]["outp"].astype(np.float32)
                  + b_out)
    return out


# revision 25
# speedup vs baseline: 1.2483x; 1.0321x over previous
"""Causal self-attention (B=4, T=2048, C=1024, H=16) on 8 TRN2 NeuronCores.

Sharding: core = (batch, head-group) — data parallel over the 4 batches,
tensor parallel over 2 groups of 8 heads (Megatron-style column/row split of
the qkv / out projections).  Each core computes a [T, C] partial of the out
projection for its head group; the host sums the two partials per batch and
adds b_out, so no device collectives are needed.

v2 (vs v1 baseline at ~630us):
  * All matmul operands are bf16 (PSUM accumulation stays fp32).  Same PE
    streaming rate as f32r (1 cycle/row) but enables Fast Weight Load
    (v1 spent 253us in serialized fp32 LDWEIGHTS), removes the f32r
    moving-dim<256 4x penalty, and halves DMA + SBUF footprint.  Host casts
    inputs to bf16.
  * One exp ACTIVATE per tk-block covering BOTH heads of a pair via a
    [128, 2, 512] PSUM tile spanning 2 banks (v1: 320 exps -> 160, less
    fixed per-instruction overhead on ScalarE).
  * Softmax normalization: v1 burned 107us of DVE in single-partition
    5-pass RECIPROCALs.  Now: copy the two denominator rows (PSUM row D)
    to partitions 0/1, one reciprocal_approx_fast on [2,512], one DRAM
    bounce DMA broadcasting both heads' 1/den to [64, 2, 512], then one
    tensor_mul per head.
  * Loop body stays slab-interleaved (projections / attention / out-proj)
    so the Tile scheduler can fill PE gaps during ScalarE exp latency with
    next-slab projection matmuls — keeping the PE HAM-warm at 2.4 GHz
    (v1 ran 67% of the time at the 1.2 GHz throttle).
"""

import os
import sys
from contextlib import ExitStack

import numpy as np

for _p in ("/opt/trn_rl_repo", "/root/.axon_site/_ro/trn_rl_repo"):
    if os.path.isdir(_p) and _p not in sys.path:
        sys.path.append(_p)

import concourse.bacc as bacc
import concourse.bass as bass
import concourse.tile as tile
from concourse import mybir
from concourse.bass_utils import run_bass_kernel_spmd
from concourse.masks import make_upper_triangular

AF = mybir.ActivationFunctionType
ALU = mybir.AluOpType
F32 = mybir.dt.float32
BF16 = mybir.dt.bfloat16

P = 128
SLAB = 512

B, T, C, H, D = 4, 2048, 1024, 16, 64
N_CORES = 8
N_GROUPS = 2          # head groups (tensor-parallel degree per batch)
HL = H // N_GROUPS    # heads per core
CL = HL * D           # local qkv width


def _build_nc():
    NCK = C // P
    MQK = 2 * CL // P
    MQ = MQK // 2
    TT = T // P
    NS = T // SLAB
    YC = CL // P
    W_OUT = min(SLAB, C)
    NOUT = C // W_OUT
    scale = 1.0 / np.sqrt(D)

    nc = bacc.Bacc("TRN2", target_bir_lowering=False, debug=False,
                   num_devices=N_CORES)
    xT = nc.dram_tensor("xT", [C, T], BF16, kind="ExternalInput")
    wqk = nc.dram_tensor("wqk", [C, 2 * CL], BF16, kind="ExternalInput")
    wv = nc.dram_tensor("wv", [C, CL], BF16, kind="ExternalInput")
    wout = nc.dram_tensor("wout", [CL, C], BF16, kind="ExternalInput")
    bqk = nc.dram_tensor("bqk", [P, MQK], F32, kind="ExternalInput")
    outp = nc.dram_tensor("outp", [T, C], BF16, kind="ExternalOutput")
    scr = nc.dram_tensor("scr", [2 * HL // 2 * NS, SLAB], BF16)

    with tile.TileContext(nc) as tc, ExitStack() as ctx:
        pool = lambda name, bufs, **kw: ctx.enter_context(
            tc.tile_pool(name=name, bufs=bufs, **kw))

        const = pool("const", 1)
        kp = pool("kp", 1)
        vp = pool("vp", 1)
        wqkp = pool("wqkp", 1)
        wvp = pool("wvp", 1)
        woutp = pool("woutp", 1)
        xp = pool("xp", 1)
        qp = pool("qp", 2)
        yTp = pool("yTp", 2)
        expp = pool("expp", 3)
        yrawp = pool("yrawp", 2)
        den2p = pool("den2p", 2)
        recbp = pool("recbp", 2)
        y8p = pool("y8", 2)
        otp = pool("ot", 2)
        psS = pool("psS", 2, space="PSUM")      # [P,2,SLAB] pair tiles: 4 banks
        psY = pool("psY", 1, space="PSUM")      # py0+py1: 2 banks
        psPO = pool("psPO", 2, space="PSUM")    # shared proj/out evict: 2 banks

        k_sb = kp.tile([P, MQ, T], BF16)
        v_sb = vp.tile([P, TT, HL, D + 1], BF16)
        x_sb = xp.tile([P, NCK, T], BF16)
        wqk_sb = wqkp.tile([P, NCK, 2 * CL], BF16)
        wv_sb = wvp.tile([P, NCK, CL], BF16)
        wout_sb = woutp.tile([P, YC, C], BF16)
        bqk_sb = const.tile([P, MQK], F32)
        mask01 = const.tile([P, P], BF16)
        maskf = const.tile([P, P], F32)

        # Preload: sync queue carries what phase A(0) needs (wqk + the
        # first half of x), interleaved so chunk 0 of each lands early;
        # x rows stay >=2KB contiguous (per-slab [128,512] slices would be
        # 1KB strided rows, which the DMA engines move ~4x slower).
        # Everything else goes through the gpsimd SWDGE queue so the
        # ~0.6us per-DMA issue cost doesn't serialize the first matmuls.
        TH = T // 2
        for c in range(NCK):
            nc.sync.dma_start(out=wqk_sb[:, c, :], in_=wqk[c * P:(c + 1) * P, :])
            nc.sync.dma_start(out=x_sb[:, c, 0:TH],
                              in_=xT[c * P:(c + 1) * P, 0:TH])
        nc.gpsimd.dma_start(out=bqk_sb[:, :], in_=bqk[:, :])
        for c in range(NCK):
            nc.gpsimd.dma_start(out=wv_sb[:, c, :], in_=wv[c * P:(c + 1) * P, :])
        for c in range(NCK):
            nc.gpsimd.dma_start(out=x_sb[:, c, TH:T],
                                in_=xT[c * P:(c + 1) * P, TH:T])
        for c in range(YC):
            nc.gpsimd.dma_start(out=wout_sb[:, c, :],
                                in_=wout[c * P:(c + 1) * P, :])
        # mask01[p, f] = 1 if f >= p else 0  (S^T visibility: tq >= tk);
        # built in f32 then cast (affine_select path is f32).
        make_upper_triangular(nc, maskf[:, :], val=1.0, diag=True)
        nc.vector.tensor_copy(mask01[:, :], maskf[:, :])
        nc.vector.memset(v_sb[:, :, :, D], 1.0)

        for s in range(NS):
            t0 = s * SLAB
            # ---- A(s): projections for this slab ----
            q_sb = qp.tile([P, MQ, SLAB], BF16)
            for m in range(MQK):
                ps = psPO.tile([P, SLAB], F32, tag="po")
                for c in range(NCK):
                    nc.tensor.matmul(
                        ps[:, :],
                        wqk_sb[:, c, m * P:(m + 1) * P],
                        x_sb[:, c, t0:t0 + SLAB],
                        start=(c == 0), stop=(c == NCK - 1))
                dst = (q_sb[:, m, :] if m < MQ
                       else k_sb[:, m - MQ, t0:t0 + SLAB])
                sc = scale if m < MQ else 1.0
                nc.vector.tensor_scalar(
                    dst, ps[:, :], sc, bqk_sb[:, m:m + 1],
                    op0=ALU.mult, op1=ALU.add)
            # v projection WITHOUT bias: since softmax rows sum to 1, the
            # v-bias passes through attention additively and is folded
            # into b_out on the host (b_out + b_v @ W_out).
            for sub in range(SLAB // P):
                tt = s * (SLAB // P) + sub
                ps = psPO.tile([P, CL], F32, tag="po")
                for c in range(NCK):
                    nc.tensor.matmul(
                        ps[:, :],
                        x_sb[:, c, tt * P:(tt + 1) * P],
                        wv_sb[:, c, :],
                        start=(c == 0), stop=(c == NCK - 1))
                nc.vector.tensor_copy(
                    v_sb[:, tt, :, 0:D],
                    ps[:, :].rearrange("p (h d) -> p h d", d=D))

            # ---- B(s): attention; even/odd head pairs share the PE array
            # via tile_position row groups (concurrent K=64).  One exp per
            # tk-block covers both heads ([P, 2, SLAB] across 2 PSUM banks).
            # C(s-1) out-projection groups are interleaved after each pair:
            # Tile's schedule is STATIC and each engine queue is strict
            # FIFO, so C(s) emitted right after B(s) would block the PE
            # behind the last pair's normalize/bounce chain; C(s-1)'s
            # inputs are long complete and give the PE filler work.
            yT_prev = yT_sb if s > 0 else None
            t0_prev = t0 - SLAB
            yT_sb = yTp.tile([P, YC, SLAB], BF16)
            for hp in range(HL // 2):
                nblk = (s + 1) * SLAB // P
                py0 = psY.tile([D + 1, SLAB], F32, tag="py0")
                py1 = psY.tile([D + 1, SLAB], F32, tag="py1")
                pys = (py0, py1)
                for b in range(nblk):
                    tk0 = b * P
                    off = tk0 - t0
                    vis = max(0, off)
                    ps3 = psS.tile([P, 2, SLAB], F32)
                    for i in range(2):
                        row0 = i * 64
                        nc.tensor.matmul(
                            ps3[:, i, vis:SLAB],
                            k_sb[row0:row0 + 64, hp, tk0:tk0 + P],
                            q_sb[row0:row0 + 64, hp, vis:SLAB],
                            start=True, stop=True,
                            tile_position=(row0, 0))
                    ep3 = expp.tile([P, 2, SLAB], BF16)
                    nc.scalar.activation(ep3[:, :, vis:SLAB],
                                         ps3[:, :, vis:SLAB], AF.Exp)
                    if off >= 0:
                        for i in range(2):
                            nc.gpsimd.tensor_mul(
                                ep3[:, i, off:off + P],
                                ep3[:, i, off:off + P], mask01[:, :])
                    for i in range(2):
                        nc.tensor.matmul(
                            pys[i][0:D + 1, vis:SLAB],
                            v_sb[:, b, 2 * hp + i, 0:D + 1],
                            ep3[:, i, vis:SLAB],
                            start=(b == 0), stop=(b == nblk - 1))
                # Evict py0/py1 RAW to SBUF immediately (ScalarE + DVE in
                # parallel) so the psY banks free up and the next pair's PV
                # can start; the whole normalize chain then runs from SBUF
                # off the PE critical path.  (reciprocal_approx_fast needs
                # an SBUF source — PSUM reads feed it garbage.)
                yraw0 = yrawp.tile([D + 1, SLAB], F32, tag="yraw0")
                yraw1 = yrawp.tile([D + 1, SLAB], F32, tag="yraw1")
                nc.scalar.copy(yraw0[:, :], py0[:, :])
                nc.vector.tensor_copy(yraw1[:, :], py1[:, :])
                yraws = (yraw0, yraw1)
                sidx = 2 * (hp * NS + s)
                # 1/den first (reciprocal_approx_fast needs an SBUF source
                # at partition base 0; ScalarE handles the partition-64
                # row move), cast to bf16, then a DRAM bounce broadcasts
                # both heads' 1/den across partitions in one 128KB DMA.
                den2 = den2p.tile([1, 2 * SLAB], F32, tag="den2")
                rec2 = den2p.tile([1, 2 * SLAB], F32, tag="rec2")
                recc = den2p.tile([1, 2 * SLAB], BF16, tag="recc")
                # ScalarE is exp-saturated in the later (bigger) slabs;
                # route the den row moves to DVE there.
                den_cp = (nc.vector.tensor_copy if s >= 2
                          else nc.scalar.copy)
                for i in range(2):
                    den_cp(den2[:, i * SLAB:(i + 1) * SLAB],
                           yraws[i][D:D + 1, :])
                nc.vector.reciprocal_approx_fast(rec2[:, :], den2[:, :])
                nc.vector.tensor_copy(recc[:, :], rec2[:, :])
                nc.sync.dma_start(out=scr[sidx:sidx + 2, :],
                                  in_=recc[:, :])
                src = scr[sidx:sidx + 2, :]
                bsrc = bass.AP(tensor=src.tensor, offset=src.offset,
                               ap=[[0, 64], [SLAB, 2], [1, SLAB]])
                recb = recbp.tile([64, 2, SLAB], BF16, tag="recb")
                nc.sync.dma_start(out=recb[:, :, :], in_=bsrc)
                y8 = y8p.tile([P, SLAB], BF16)
                for i in range(2):
                    nc.vector.tensor_mul(y8[i * 64:(i + 1) * 64, :],
                                         yraws[i][0:D, :], recb[:, i, :])
                nc.sync.dma_start(out=yT_sb[:, hp, :], in_=y8[:, :])

                # ---- C(s-1) filler: two out-projection groups per pair --
                if yT_prev is not None:
                    for g in range(2):
                        sub, n = divmod(2 * hp + g, NOUT)
                        n0 = n * W_OUT
                        ps = psPO.tile([P, W_OUT], F32, tag="po")
                        for c in range(YC):
                            nc.tensor.matmul(
                                ps[:, :],
                                yT_prev[:, c, sub * P:(sub + 1) * P],
                                wout_sb[:, c, n0:n0 + W_OUT],
                                start=(c == 0), stop=(c == YC - 1))
                        ot = otp.tile([P, W_OUT], BF16)
                        nc.vector.tensor_copy(ot[:, :], ps[:, :])
                        nc.sync.dma_start(
                            out=outp[t0_prev + sub * P:
                                     t0_prev + (sub + 1) * P,
                                     n0:n0 + W_OUT],
                            in_=ot[:, :])

        # ---- C(NS-1): final slab's out projection (tail) ----
        t0 = (NS - 1) * SLAB
        for sub in range(SLAB // P):
            for n in range(NOUT):
                n0 = n * W_OUT
                ps = psPO.tile([P, W_OUT], F32, tag="po")
                for c in range(YC):
                    nc.tensor.matmul(
                        ps[:, :],
                        yT_sb[:, c, sub * P:(sub + 1) * P],
                        wout_sb[:, c, n0:n0 + W_OUT],
                        start=(c == 0), stop=(c == YC - 1))
                ot = otp.tile([P, W_OUT], BF16)
                nc.vector.tensor_copy(ot[:, :], ps[:, :])
                nc.sync.dma_start(
                    out=outp[t0 + sub * P:t0 + (sub + 1) * P,
                             n0:n0 + W_OUT],
                    in_=ot[:, :])

    nc.compile()
    return nc


_NC_CACHE = None


def _get_nc():
    global _NC_CACHE
    if _NC_CACHE is None:
        _NC_CACHE = _build_nc()
    return _NC_CACHE


def make_in_maps(x, W_qkv, b_qkv, W_out):
    bf16 = mybir.dt.np(BF16)
    scale = 1.0 / np.sqrt(D)
    MQK = 2 * CL // P
    in_maps = []
    for core in range(N_CORES):
        b, hg = divmod(core, N_GROUPS)
        qs = slice(hg * CL, (hg + 1) * CL)
        ks = slice(C + hg * CL, C + (hg + 1) * CL)
        vs = slice(2 * C + hg * CL, 2 * C + (hg + 1) * CL)
        bqk_cat = np.concatenate([b_qkv[qs] * scale, b_qkv[ks]])
        in_maps.append({
            "xT": np.ascontiguousarray(x[b].T).astype(bf16),
            "wqk": np.ascontiguousarray(
                np.concatenate([W_qkv[:, qs], W_qkv[:, ks]],
                               axis=1)).astype(bf16),
            "wv": np.ascontiguousarray(W_qkv[:, vs]).astype(bf16),
            "wout": np.ascontiguousarray(W_out[hg * CL:(hg + 1) * CL,
                                               :]).astype(bf16),
            "bqk": np.ascontiguousarray(bqk_cat.reshape(MQK, P).T),
        })
    return in_maps


def kernel(x, W_qkv, b_qkv, W_out, b_out):
    x = np.asarray(x, dtype=np.float32)
    W_qkv = np.asarray(W_qkv, dtype=np.float32)
    b_qkv = np.asarray(b_qkv, dtype=np.float32)
    W_out = np.asarray(W_out, dtype=np.float32)
    b_out = np.asarray(b_out, dtype=np.float32)

    nc = _get_nc()
    in_maps = make_in_maps(x, W_qkv, b_qkv, W_out)
    res = run_bass_kernel_spmd(nc, in_maps, core_ids=list(range(N_CORES)))

    # softmax rows sum to 1, so the v-bias contributes b_v @ W_out to
    # every output row — folded here instead of on-device.
    b_eff = b_out + b_qkv[2 * C:] @ W_out
    out = np.empty((B, T, C), dtype=np.float32)
    for b in range(B):
        out[b] = (res.results[N_GROUPS * b]["outp"].astype(np.float32)
                  + res.results[N_GROUPS * b + 1]["outp"].astype(np.float32)
                  + b_eff)
    return out


# revision 27
# speedup vs baseline: 1.3322x; 1.0672x over previous
"""Causal self-attention (B=4, T=2048, C=1024, H=16) on 8 TRN2 NeuronCores.

Sharding: core = (batch, head-group) — data parallel over the 4 batches,
tensor parallel over 2 groups of 8 heads (Megatron-style column/row split of
the qkv / out projections).  Each core computes a [T, C] partial of the out
projection for its head group; the host sums the two partials per batch and
adds b_out, so no device collectives are needed.

v2 (vs v1 baseline at ~630us):
  * All matmul operands are bf16 (PSUM accumulation stays fp32).  Same PE
    streaming rate as f32r (1 cycle/row) but enables Fast Weight Load
    (v1 spent 253us in serialized fp32 LDWEIGHTS), removes the f32r
    moving-dim<256 4x penalty, and halves DMA + SBUF footprint.  Host casts
    inputs to bf16.
  * One exp ACTIVATE per tk-block covering BOTH heads of a pair via a
    [128, 2, 512] PSUM tile spanning 2 banks (v1: 320 exps -> 160, less
    fixed per-instruction overhead on ScalarE).
  * Softmax normalization: v1 burned 107us of DVE in single-partition
    5-pass RECIPROCALs.  Now: copy the two denominator rows (PSUM row D)
    to partitions 0/1, one reciprocal_approx_fast on [2,512], one DRAM
    bounce DMA broadcasting both heads' 1/den to [64, 2, 512], then one
    tensor_mul per head.
  * Loop body stays slab-interleaved (projections / attention / out-proj)
    so the Tile scheduler can fill PE gaps during ScalarE exp latency with
    next-slab projection matmuls — keeping the PE HAM-warm at 2.4 GHz
    (v1 ran 67% of the time at the 1.2 GHz throttle).
"""

import os
import sys
from contextlib import ExitStack

import numpy as np

for _p in ("/opt/trn_rl_repo", "/root/.axon_site/_ro/trn_rl_repo"):
    if os.path.isdir(_p) and _p not in sys.path:
        sys.path.append(_p)

import concourse.bacc as bacc
import concourse.bass as bass
import concourse.tile as tile
from concourse import mybir
from concourse.bass_utils import run_bass_kernel_spmd
from concourse.masks import make_upper_triangular

AF = mybir.ActivationFunctionType
ALU = mybir.AluOpType
F32 = mybir.dt.float32
BF16 = mybir.dt.bfloat16

P = 128
SLAB = 512

B, T, C, H, D = 4, 2048, 1024, 16, 64
N_CORES = 8
N_GROUPS = 2          # head groups (tensor-parallel degree per batch)
HL = H // N_GROUPS    # heads per core
CL = HL * D           # local qkv width


def _build_nc():
    NCK = C // P
    MQK = 2 * CL // P
    MQ = MQK // 2
    TT = T // P
    NS = T // SLAB
    YC = CL // P
    W_OUT = min(SLAB, C)
    NOUT = C // W_OUT
    scale = 1.0 / np.sqrt(D)

    nc = bacc.Bacc("TRN2", target_bir_lowering=False, debug=False,
                   num_devices=N_CORES)
    xT = nc.dram_tensor("xT", [C, T], BF16, kind="ExternalInput")
    wqk = nc.dram_tensor("wqk", [C, 2 * CL], BF16, kind="ExternalInput")
    wv = nc.dram_tensor("wv", [C, CL], BF16, kind="ExternalInput")
    wout = nc.dram_tensor("wout", [CL, C], BF16, kind="ExternalInput")
    bqk = nc.dram_tensor("bqk", [P, MQK], F32, kind="ExternalInput")
    outp = nc.dram_tensor("outp", [T, C], BF16, kind="ExternalOutput")
    scr = nc.dram_tensor("scr", [2 * HL // 2 * NS, SLAB], BF16)

    with tile.TileContext(nc) as tc, ExitStack() as ctx:
        pool = lambda name, bufs, **kw: ctx.enter_context(
            tc.tile_pool(name=name, bufs=bufs, **kw))

        const = pool("const", 1)
        kp = pool("kp", 1)
        vp = pool("vp", 1)
        wqkp = pool("wqkp", 1)
        wvp = pool("wvp", 1)
        woutp = pool("woutp", 1)
        xp = pool("xp", 1)
        qp = pool("qp", 2)
        yTp = pool("yTp", 2)
        expp = pool("expp", 4)
        yrawp = pool("yrawp", 2)
        den2p = pool("den2p", 2)
        recbp = pool("recbp", 2)
        y8p = pool("y8", 2)
        otp = pool("ot", 2)
        psS = pool("psS", 2, space="PSUM")      # [P,2,SLAB] pair tiles: 4 banks
        psY = pool("psY", 1, space="PSUM")      # py0+py1: 2 banks
        psPO = pool("psPO", 2, space="PSUM")    # shared proj/out evict: 2 banks

        k_sb = kp.tile([P, MQ, T], BF16)
        v_sb = vp.tile([P, TT, HL, D + 1], BF16)
        x_sb = xp.tile([P, NCK, T], BF16)
        wqk_sb = wqkp.tile([P, NCK, 2 * CL], BF16)
        wv_sb = wvp.tile([P, NCK, CL], BF16)
        wout_sb = woutp.tile([P, YC, C], BF16)
        bqk_sb = const.tile([P, MQK], F32)
        mask01 = const.tile([P, P], BF16)
        maskf = const.tile([P, P], F32)

        # Preload: sync queue carries what phase A(0) needs (wqk + the
        # first half of x), interleaved so chunk 0 of each lands early;
        # x rows stay >=2KB contiguous (per-slab [128,512] slices would be
        # 1KB strided rows, which the DMA engines move ~4x slower).
        # Everything else goes through the gpsimd SWDGE queue so the
        # ~0.6us per-DMA issue cost doesn't serialize the first matmuls.
        TH = T // 2
        for c in range(NCK):
            nc.sync.dma_start(out=wqk_sb[:, c, :], in_=wqk[c * P:(c + 1) * P, :])
            nc.sync.dma_start(out=x_sb[:, c, 0:TH],
                              in_=xT[c * P:(c + 1) * P, 0:TH])
        nc.gpsimd.dma_start(out=bqk_sb[:, :], in_=bqk[:, :])
        for c in range(NCK):
            nc.gpsimd.dma_start(out=wv_sb[:, c, :], in_=wv[c * P:(c + 1) * P, :])
        for c in range(NCK):
            nc.gpsimd.dma_start(out=x_sb[:, c, TH:T],
                                in_=xT[c * P:(c + 1) * P, TH:T])
        for c in range(YC):
            nc.gpsimd.dma_start(out=wout_sb[:, c, :],
                                in_=wout[c * P:(c + 1) * P, :])
        # mask01[p, f] = 1 if f >= p else 0  (S^T visibility: tq >= tk);
        # built in f32 then cast (affine_select path is f32).
        make_upper_triangular(nc, maskf[:, :], val=1.0, diag=True)
        nc.vector.tensor_copy(mask01[:, :], maskf[:, :])
        nc.vector.memset(v_sb[:, :, :, D], 1.0)

        for s in range(NS):
            t0 = s * SLAB
            # ---- A(s): projections for this slab ----
            q_sb = qp.tile([P, MQ, SLAB], BF16)
            for m in range(MQK):
                ps = psPO.tile([P, SLAB], F32, tag="po")
                for c in range(NCK):
                    nc.tensor.matmul(
                        ps[:, :],
                        wqk_sb[:, c, m * P:(m + 1) * P],
                        x_sb[:, c, t0:t0 + SLAB],
                        start=(c == 0), stop=(c == NCK - 1))
                dst = (q_sb[:, m, :] if m < MQ
                       else k_sb[:, m - MQ, t0:t0 + SLAB])
                sc = scale if m < MQ else 1.0
                nc.vector.tensor_scalar(
                    dst, ps[:, :], sc, bqk_sb[:, m:m + 1],
                    op0=ALU.mult, op1=ALU.add)
            # v projection WITHOUT bias: since softmax rows sum to 1, the
            # v-bias passes through attention additively and is folded
            # into b_out on the host (b_out + b_v @ W_out).
            for sub in range(SLAB // P):
                tt = s * (SLAB // P) + sub
                ps = psPO.tile([P, CL], F32, tag="po")
                for c in range(NCK):
                    nc.tensor.matmul(
                        ps[:, :],
                        x_sb[:, c, tt * P:(tt + 1) * P],
                        wv_sb[:, c, :],
                        start=(c == 0), stop=(c == NCK - 1))
                nc.vector.tensor_copy(
                    v_sb[:, tt, :, 0:D],
                    ps[:, :].rearrange("p (h d) -> p h d", d=D))

            # ---- B(s): attention; even/odd head pairs share the PE array
            # via tile_position row groups (concurrent K=64).  One exp per
            # tk-block covers both heads ([P, 2, SLAB] across 2 PSUM banks).
            # C(s-1) out-projection groups are interleaved after each pair:
            # Tile's schedule is STATIC and each engine queue is strict
            # FIFO, so C(s) emitted right after B(s) would block the PE
            # behind the last pair's normalize/bounce chain; C(s-1)'s
            # inputs are long complete and give the PE filler work.
            yT_prev = yT_sb if s > 0 else None
            t0_prev = t0 - SLAB
            yT_sb = yTp.tile([P, YC, SLAB], BF16)
            for hp in range(HL // 2):
                nblk = (s + 1) * SLAB // P
                py0 = psY.tile([D + 1, SLAB], F32, tag="py0")
                py1 = psY.tile([D + 1, SLAB], F32, tag="py1")
                pys = (py0, py1)
                for b in range(nblk):
                    tk0 = b * P
                    off = tk0 - t0
                    vis = max(0, off)
                    ps3 = psS.tile([P, 2, SLAB], F32)
                    for i in range(2):
                        row0 = i * 64
                        nc.tensor.matmul(
                            ps3[:, i, vis:SLAB],
                            k_sb[row0:row0 + 64, hp, tk0:tk0 + P],
                            q_sb[row0:row0 + 64, hp, vis:SLAB],
                            start=True, stop=True,
                            tile_position=(row0, 0))
                    ep3 = expp.tile([P, 2, SLAB], BF16)
                    nc.scalar.activation(ep3[:, :, vis:SLAB],
                                         ps3[:, :, vis:SLAB], AF.Exp)
                    if off >= 0:
                        for i in range(2):
                            nc.vector.tensor_mul(
                                ep3[:, i, off:off + P],
                                ep3[:, i, off:off + P], mask01[:, :])
                    for i in range(2):
                        nc.tensor.matmul(
                            pys[i][0:D + 1, vis:SLAB],
                            v_sb[:, b, 2 * hp + i, 0:D + 1],
                            ep3[:, i, vis:SLAB],
                            start=(b == 0), stop=(b == nblk - 1))
                # Evict py0/py1 RAW to SBUF immediately (ScalarE + DVE in
                # parallel) so the psY banks free up and the next pair's PV
                # can start; the whole normalize chain then runs from SBUF
                # off the PE critical path.  (reciprocal_approx_fast needs
                # an SBUF source — PSUM reads feed it garbage.)
                yraw0 = yrawp.tile([D + 1, SLAB], F32, tag="yraw0")
                yraw1 = yrawp.tile([D + 1, SLAB], F32, tag="yraw1")
                nc.scalar.copy(yraw0[:, :], py0[:, :])
                nc.vector.tensor_copy(yraw1[:, :], py1[:, :])
                yraws = (yraw0, yraw1)
                sidx = 2 * (hp * NS + s)
                # 1/den first (reciprocal_approx_fast needs an SBUF source
                # at partition base 0; ScalarE handles the partition-64
                # row move), cast to bf16, then a DRAM bounce broadcasts
                # both heads' 1/den across partitions in one 128KB DMA.
                den2 = den2p.tile([1, 2 * SLAB], F32, tag="den2")
                rec2 = den2p.tile([1, 2 * SLAB], F32, tag="rec2")
                recc = den2p.tile([1, 2 * SLAB], BF16, tag="recc")
                # ScalarE is exp-saturated in the later (bigger) slabs;
                # route the den row moves to DVE there.
                den_cp = (nc.vector.tensor_copy if s >= 2
                          else nc.scalar.copy)
                for i in range(2):
                    den_cp(den2[:, i * SLAB:(i + 1) * SLAB],
                           yraws[i][D:D + 1, :])
                nc.vector.reciprocal_approx_fast(rec2[:, :], den2[:, :])
                nc.vector.tensor_copy(recc[:, :], rec2[:, :])
                nc.sync.dma_start(out=scr[sidx:sidx + 2, :],
                                  in_=recc[:, :])
                src = scr[sidx:sidx + 2, :]
                bsrc = bass.AP(tensor=src.tensor, offset=src.offset,
                               ap=[[0, 64], [SLAB, 2], [1, SLAB]])
                recb = recbp.tile([64, 2, SLAB], BF16, tag="recb")
                nc.sync.dma_start(out=recb[:, :, :], in_=bsrc)
                y8 = y8p.tile([P, SLAB], BF16)
                for i in range(2):
                    nc.vector.tensor_mul(y8[i * 64:(i + 1) * 64, :],
                                         yraws[i][0:D, :], recb[:, i, :])
                nc.sync.dma_start(out=yT_sb[:, hp, :], in_=y8[:, :])

                # ---- C(s-1) filler: two out-projection groups per pair --
                if yT_prev is not None:
                    for g in range(2):
                        sub, n = divmod(2 * hp + g, NOUT)
                        n0 = n * W_OUT
                        ps = psPO.tile([P, W_OUT], F32, tag="po")
                        for c in range(YC):
                            nc.tensor.matmul(
                                ps[:, :],
                                yT_prev[:, c, sub * P:(sub + 1) * P],
                                wout_sb[:, c, n0:n0 + W_OUT],
                                start=(c == 0), stop=(c == YC - 1))
                        ot = otp.tile([P, W_OUT], BF16)
                        nc.vector.tensor_copy(ot[:, :], ps[:, :])
                        nc.sync.dma_start(
                            out=outp[t0_prev + sub * P:
                                     t0_prev + (sub + 1) * P,
                                     n0:n0 + W_OUT],
                            in_=ot[:, :])

        # ---- C(NS-1): final slab's out projection (tail) ----
        t0 = (NS - 1) * SLAB
        for sub in range(SLAB // P):
            for n in range(NOUT):
                n0 = n * W_OUT
                ps = psPO.tile([P, W_OUT], F32, tag="po")
                for c in range(YC):
                    nc.tensor.matmul(
                        ps[:, :],
                        yT_sb[:, c, sub * P:(sub + 1) * P],
                        wout_sb[:, c, n0:n0 + W_OUT],
                        start=(c == 0), stop=(c == YC - 1))
                ot = otp.tile([P, W_OUT], BF16)
                nc.vector.tensor_copy(ot[:, :], ps[:, :])
                nc.sync.dma_start(
                    out=outp[t0 + sub * P:t0 + (sub + 1) * P,
                             n0:n0 + W_OUT],
                    in_=ot[:, :])

    nc.compile()
    return nc


_NC_CACHE = None


def _get_nc():
    global _NC_CACHE
    if _NC_CACHE is None:
        _NC_CACHE = _build_nc()
    return _NC_CACHE


def make_in_maps(x, W_qkv, b_qkv, W_out):
    bf16 = mybir.dt.np(BF16)
    scale = 1.0 / np.sqrt(D)
    MQK = 2 * CL // P
    in_maps = []
    for core in range(N_CORES):
        b, hg = divmod(core, N_GROUPS)
        qs = slice(hg * CL, (hg + 1) * CL)
        ks = slice(C + hg * CL, C + (hg + 1) * CL)
        vs = slice(2 * C + hg * CL, 2 * C + (hg + 1) * CL)
        bqk_cat = np.concatenate([b_qkv[qs] * scale, b_qkv[ks]])
        in_maps.append({
            "xT": np.ascontiguousarray(x[b].T).astype(bf16),
            "wqk": np.ascontiguousarray(
                np.concatenate([W_qkv[:, qs], W_qkv[:, ks]],
                               axis=1)).astype(bf16),
            "wv": np.ascontiguousarray(W_qkv[:, vs]).astype(bf16),
            "wout": np.ascontiguousarray(W_out[hg * CL:(hg + 1) * CL,
                                               :]).astype(bf16),
            "bqk": np.ascontiguousarray(bqk_cat.reshape(MQK, P).T),
        })
    return in_maps


def kernel(x, W_qkv, b_qkv, W_out, b_out):
    x = np.asarray(x, dtype=np.float32)
    W_qkv = np.asarray(W_qkv, dtype=np.float32)
    b_qkv = np.asarray(b_qkv, dtype=np.float32)
    W_out = np.asarray(W_out, dtype=np.float32)
    b_out = np.asarray(b_out, dtype=np.float32)

    nc = _get_nc()
    in_maps = make_in_maps(x, W_qkv, b_qkv, W_out)
    res = run_bass_kernel_spmd(nc, in_maps, core_ids=list(range(N_CORES)))

    # softmax rows sum to 1, so the v-bias contributes b_v @ W_out to
    # every output row — folded here instead of on-device.
    b_eff = b_out + b_qkv[2 * C:] @ W_out
    out = np.empty((B, T, C), dtype=np.float32)
    for b in range(B):
        out[b] = (res.results[N_GROUPS * b]["outp"].astype(np.float32)
                  + res.results[N_GROUPS * b + 1]["outp"].astype(np.float32)
                  + b_eff)
    return out
